# revision 23
# baseline (speedup 1.0000x reference)
"""Trainium2 Bass kernel for nn_Cffn_68478958568093 (dense_mlp).

out = x @ U_w.T + V(z),  z = a0 + continued_fraction(a[..,1:]),
a = (sigmoid(x @ gate_w.T) * x) @ ladder_w.T

Distribution: data-parallel over the 8192 tokens across 8 NeuronCores
(1024 tokens/core), weights replicated.  All on-chip compute is in
feature-major (transposed) layout; the host transposes per-core shards in
and the final output back.

Precision/performance scheme (vs a 3-pass fp16 hi/lo baseline at 479us):
the continued fraction amplifies errors in `a` by ~1e4, so the gate and
ladder matmuls need well beyond fp16-single accuracy.  Each runs an fp16
hi*hi pass plus fp8e4m3 cross terms in the PE's DoubleRow perf mode
(2 contraction rows per cycle; dual-fp8 ldweights needs the stationary
free size to be a multiple of 32, hence the ladder tiles pad 18->32):
 - gate: xhi16 @ Whi16  +  2^-15 * (x8 @ fp8((W-Whi)*2^15))      [2.5 passes-equiv]
 - ladder: token-major (stationary = gated token block, moving = the
   18-wide ladder weights) so each matmul costs 18/9 cycles instead of
   512, `a` lands in psum already transposed for the CF, and both fp8
   cross terms are kept.  Whole phase ~1.5us.
 - U path: three fp8 DoubleRow passes x8@u8 + x8@ul8 + xu8@u8 at a
   common scale of 16 (x and U each split fp8 hi+lo, lo*lo dropped,
   ~2^-8-grade) - 3/4 the cost of one fp16 pass, dense l2 stays ~2e-3.
 - V (K=3): exact-fp32 DVE FMAs against broadcast z rows fused with the
   U-psum drain; V*16 matches the psum scale and the host divides it out.

Measured on-device: max rel err 5.2e-3, l2 2.4e-3 (tolerance 2e-2);
CoreSim 241.9us vs 478.8us baseline (1.98x), PE busy 92%.

Schedule notes: weight streams ride the ACT queue, x shards and output
ride SP, with the first x chunks split small and staggered across both
queues to cut the cold-start; u8/ul8 tiles prefetch on SP during phase A;
z broadcasts via a DRAM round trip into three separate row tiles so the
first output epilogue only waits on row 0; the last gate tile computes
its halves sequentially so its epilogue hides under its own matmuls; the
last two output tiles precompute their V part so only a psum-add and the
store remain after the final U matmuls.
"""

import numpy as np
import ml_dtypes
from contextlib import ExitStack

import concourse.bass as bass
import concourse.bacc as bacc
import concourse.mybir as mybir
import concourse.tile as tile
from concourse.bass_utils import run_bass_kernel_spmd
from concourse.masks import make_identity

NCORES = 8
D = 2048
TOKENS = 4 * 2048
TPC = TOKENS // NCORES      # tokens per core = 1024
KT = D // 128               # 16 contraction chunks
NDT = D // 128              # 16 output-row tiles
NTT = TPC // 128            # 8 token tiles of 128
L = 3
DEPTH = 5
LK = L * (DEPTH + 1)        # 18
EPS = 0.01
S11 = 2048.0                # 2^11 lo-part scale (x, gated)
C16 = 16.0                  # 2^4 hi-part fp8 scale (weights)
S15 = 32768.0               # 2^15 lo-part fp8 scale (weights)
ALPHA = 1.0 / (S11 * C16)   # 2^-15 cross-psum combine scale
F16 = mybir.dt.float16
F8 = mybir.dt.float8e4
F32 = mybir.dt.float32
AOP = mybir.AluOpType
DR = mybir.MatmulPerfMode.DoubleRow
E4 = ml_dtypes.float8_e4m3


def _build_program():
    nc = bacc.Bacc()

    def dp(name, shape, dt, out=False):
        return nc.declare_dram_parameter(name, list(shape), dt, isOutput=out)

    d_xhi = dp("xhi", [128, KT, TPC], F16)
    d_x8 = dp("x8", [128, KT, TPC], F8)
    d_xlo8 = dp("xlo8", [128, KT, TPC], F8)       # fp8((x - xhi16) * 2^11)
    d_xu8 = dp("xu8", [128, KT, TPC], F8)         # fp8(x - x8), for U lo pass
    d_ghi = dp("ghi", [NDT, 128, KT, 128], F16)   # [dt][p][k][o]
    d_wl8 = dp("wl8", [NDT, 128, KT, 128], F8)    # fp8((W-Whi)*2^15)
    d_u8 = dp("u8", [NDT, 128, KT, 128], F8)      # fp8(U*16)
    d_ul8 = dp("ul8", [NDT, 128, KT, 128], F8)    # fp8(U*16 - u8)
    d_lwhi = dp("lwhi", [128, KT, LK], F16)
    d_lh8 = dp("lh8", [128, KT, 32], F8)          # zero-padded 18->32 for DoubleRow
    d_ll8 = dp("ll8", [128, KT, 32], F8)
    d_vw = dp("vw", [128, NDT, L], F32)           # V_w rows * 16, by partition
    d_out = dp("outT", [D, TPC], F32, out=True)   # 16x the final output

    with tile.TileContext(nc) as tc, ExitStack() as ctx:
        persist = ctx.enter_context(tc.tile_pool(name="persist", bufs=1))

        # x shards, chunked so early matmuls don't wait on the full load.
        # xhi chunks 0/2 ride SP, 1/3 ride ACT (after dt=0's weights) so the
        # first gate tile's hi matmuls aren't starved by one queue.
        xhi_sizes = [2, 2, 4, 4, 4]
        xhi_starts = [0, 2, 4, 8, 12]
        xhi = [persist.tile([128, n, TPC], F16, name=f"xhi{c}", tag=f"xhi{c}")
               for c, n in enumerate(xhi_sizes)]
        x8 = [persist.tile([128, 8, TPC], F8, name=f"x8_{c}", tag=f"x8_{c}")
              for c in range(2)]
        xlo8 = [persist.tile([128, 8, TPC], F8, name=f"xlo8_{c}", tag=f"xlo8_{c}")
                for c in range(2)]
        xu8 = [persist.tile([128, 8, TPC], F8, name=f"xu8_{c}", tag=f"xu8_{c}")
               for c in range(2)]

        def ld_x(eng, t, d, c4, n):
            eng.dma_start(out=t, in_=d[:, :, :][:, c4:c4 + n, :])

        ld_x(nc.sync, xhi[0], d_xhi, 0, 2)
        ld_x(nc.sync, xhi[1], d_xhi, 2, 2)

        def xhi_s(k, sl):
            c = 1 if 2 <= k < 4 else (0 if k < 4 else 2 + (k - 4) // 4)
            return xhi[c][:, (k - xhi_starts[c]), sl]

        def pair(tiles, k, sl):             # DoubleRow pair slice [128, 2, n]
            return tiles[k // 8][:, (k % 8):(k % 8) + 2, sl]

        lwhi = persist.tile([128, KT, LK], F16, tag="lwhi")
        lh8 = persist.tile([128, KT, 32], F8, tag="lh8")
        ll8 = persist.tile([128, KT, 32], F8, tag="ll8")
        vw = persist.tile([128, NDT, L], F32, tag="vw")
        ident = persist.tile([128, 128], F32, tag="ident")

        # gated_x: fp16 hi + fp8 copies for the ladder cross terms
        # (two 8-k halves so phase B's early matmuls don't wait on the
        # last gate epilogues)
        Ghi = [persist.tile([128, 8, TPC], F16, name=f"Ghi{h}", tag=f"Ghi{h}")
               for h in range(2)]
        G8 = [persist.tile([128, 8, TPC], F8, name=f"G8_{h}", tag=f"G8_{h}")
              for h in range(2)]
        Glo8 = [persist.tile([128, 8, TPC], F8, name=f"Glo8_{h}", tag=f"Glo8_{h}")
                for h in range(2)]
        zt = persist.tile([128, NTT, L], F32, tag="zt")

        uwp = ctx.enter_context(tc.tile_pool(name="uw", bufs=2))
        u_tiles = {}

        def load_u(dt, eng=None):
            eng = eng or nc.scalar
            u8t = uwp.tile([128, KT, 128], F8, name=f"u8t{dt}", tag="u8t", bufs=4)
            eng.dma_start(out=u8t, in_=d_u8[:, :, :, :][dt])
            ul8t = uwp.tile([128, KT, 128], F8, name=f"ul8t{dt}", tag="ul8t", bufs=4)
            eng.dma_start(out=ul8t, in_=d_ul8[:, :, :, :][dt])
            return u8t, ul8t

        # ---------------- Phase A: gated_x = sigmoid(x @ gate_w.T) * x -----
        with tc.tile_pool(name="gw", bufs=3) as gwp, \
             tc.tile_pool(name="psA", bufs=2, space="PSUM") as psA, \
             tc.tile_pool(name="epi", bufs=2) as epi:
            for dt in range(NDT):
                gh = gwp.tile([128, KT, 128], F16, tag="gh")
                nc.scalar.dma_start(out=gh, in_=d_ghi[:, :, :, :][dt])
                wl = gwp.tile([128, KT, 128], F8, tag="wl")
                nc.scalar.dma_start(out=wl, in_=d_wl8[:, :, :, :][dt])
                if dt == 0:
                    # stagger the rest of the x load across both queues
                    ld_x(nc.scalar, xhi[2], d_xhi, 4, 4)
                    ld_x(nc.scalar, xhi[4], d_xhi, 12, 4)
                    ld_x(nc.sync, xhi[3], d_xhi, 8, 4)
                    ld_x(nc.sync, x8[0], d_x8, 0, 8)
                    ld_x(nc.sync, xlo8[0], d_xlo8, 0, 8)
                    ld_x(nc.sync, x8[1], d_x8, 8, 8)
                    ld_x(nc.sync, xlo8[1], d_xlo8, 8, 8)
                    ld_x(nc.sync, xu8[0], d_xu8, 0, 8)
                    ld_x(nc.sync, xu8[1], d_xu8, 8, 8)
                    nc.sync.dma_start(out=lwhi, in_=d_lwhi[:, :, :])
                    nc.sync.dma_start(out=lh8, in_=d_lh8[:, :, :])
                    nc.sync.dma_start(out=ll8, in_=d_ll8[:, :, :])
                    nc.sync.dma_start(out=vw, in_=d_vw[:, :])
                    make_identity(nc, ident)

                if dt == 8:
                    u_tiles[0] = load_u(0, nc.sync)
                    u_tiles[1] = load_u(1, nc.sync)
                if dt == 12:
                    u_tiles[2] = load_u(2, nc.sync)
                    u_tiles[3] = load_u(3, nc.sync)

                pm = [psA.tile([128, 512], F32, name=f"pm{dt}_{t}", tag=f"pm{t}") for t in range(2)]
                pc = [psA.tile([128, 512], F32, name=f"pc{dt}_{t}", tag=f"pc{t}") for t in range(2)]

                def emit_mms(t):
                    sl = slice(t * 512, (t + 1) * 512)
                    for k in range(KT):
                        nc.tensor.matmul(pm[t], gh[:, k, :], xhi_s(k, sl),
                                         start=(k == 0), stop=(k == KT - 1))
                    for k in range(0, KT, 2):
                        nc.tensor.matmul(pc[t], wl[:, k:k + 2, :], pair(x8, k, sl),
                                         start=(k == 0), stop=(k == KT - 2),
                                         perf_mode=DR)

                if dt < NDT - 1:
                    for k in range(KT):
                        for t in range(2):
                            sl = slice(t * 512, (t + 1) * 512)
                            nc.tensor.matmul(pm[t], gh[:, k, :], xhi_s(k, sl),
                                             start=(k == 0), stop=(k == KT - 1))
                    for k in range(0, KT, 2):
                        for t in range(2):
                            sl = slice(t * 512, (t + 1) * 512)
                            nc.tensor.matmul(pc[t], wl[:, k:k + 2, :],
                                             pair(x8, k, sl),
                                             start=(k == 0), stop=(k == KT - 2),
                                             perf_mode=DR)
                else:
                    emit_mms(0)   # t=0 epilogue overlaps t=1's matmuls

                g32 = epi.tile([128, TPC], F32, tag="g32")
                sig = epi.tile([128, TPC], F32, tag="sig")
                x32 = epi.tile([128, TPC], F32, tag="x32")
                h, hk = dt // 8, dt % 8
                for t in range(2):
                    if dt == NDT - 1 and t == 1:
                        emit_mms(1)
                    sl = slice(t * 512, (t + 1) * 512)
                    nc.scalar.copy(g32[:, sl], pm[t])
                    nc.vector.scalar_tensor_tensor(
                        out=g32[:, sl], in0=pc[t], scalar=ALPHA,
                        in1=g32[:, sl], op0=AOP.mult, op1=AOP.add)
                    nc.scalar.activation(sig[:, sl], g32[:, sl],
                                         mybir.ActivationFunctionType.Sigmoid)
                    nc.vector.scalar_tensor_tensor(
                        out=x32[:, sl], in0=xlo8[h][:, hk, sl],
                        scalar=1.0 / S11, in1=xhi_s(dt, sl),
                        op0=AOP.mult, op1=AOP.add)
                    nc.vector.tensor_mul(g32[:, sl], sig[:, sl], x32[:, sl])
                    nc.scalar.copy(Ghi[h][:, hk, sl], g32[:, sl])
                    nc.vector.scalar_tensor_tensor(     # resid -> x32 (reuse)
                        out=x32[:, sl], in0=Ghi[h][:, hk, sl], scalar=-1.0,
                        in1=g32[:, sl], op0=AOP.mult, op1=AOP.add)
                    nc.scalar.activation(Glo8[h][:, hk, sl], x32[:, sl],
                                         mybir.ActivationFunctionType.Copy,
                                         scale=S11)
                    nc.scalar.copy(G8[h][:, hk, sl], g32[:, sl])

        # ---------------- Phase B: a = gated @ lw.T ; CF ; z ---------------
        drp = ctx.enter_context(tc.tile_pool(name="drs", bufs=1, space="DRAM"))
        with tc.tile_pool(name="cfb", bufs=1) as cfb, \
             tc.tile_pool(name="psB", bufs=2, space="PSUM") as psB:
            zT32 = cfb.tile([L, TPC], F32, tag="zT32")
            # token-major: stationary = gated token block, moving = the
            # 18-wide ladder weights, so `a` lands in psum already
            # transposed and each matmul costs only 18 (9 for DR) cycles.
            at = cfb.tile([128, NTT, L, DEPTH + 1], F32, tag="at")
            for tt in range(NTT):
                tsl = slice(tt * 128, (tt + 1) * 128)
                pam = psB.tile([128, LK], F32, tag="pam")
                pac = psB.tile([128, LK], F32, tag="pac")
                for k in range(KT):
                    nc.tensor.matmul(pam, Ghi[k // 8][:, k % 8, tsl],
                                     lwhi[:, k, :],
                                     start=(k == 0), stop=(k == KT - 1))
                for k in range(0, KT, 2):
                    nc.tensor.matmul(pac, pair(G8, k, tsl),
                                     ll8[:, k:k + 2, :LK],
                                     start=(k == 0), stop=False, perf_mode=DR)
                    nc.tensor.matmul(pac, pair(Glo8, k, tsl),
                                     lh8[:, k:k + 2, :LK],
                                     start=False, stop=(k == KT - 2), perf_mode=DR)
                av = at[:, tt, :, :].rearrange("p l k -> p (l k)")
                nc.scalar.copy(av, pam)
                nc.vector.scalar_tensor_tensor(
                    out=av, in0=pac, scalar=ALPHA, in1=av,
                    op0=AOP.mult, op1=AOP.add)

            # continued fraction with eps-guarded denominators
            f = cfb.tile([128, NTT, L], F32, tag="f")
            t1 = cfb.tile([128, NTT, L], F32, tag="t1")
            dc = cfb.tile([128, NTT, L], F32, tag="dc")
            msk = cfb.tile([128, NTT, L], mybir.dt.uint8, tag="msk")
            rc = cfb.tile([128, NTT, L], F32, tag="rc")
            nc.vector.tensor_copy(f, at[:, :, :, DEPTH])
            for kk in range(DEPTH - 1, 0, -1):
                nc.vector.tensor_scalar(out=t1, in0=f, scalar1=1.0,
                                        scalar2=EPS, op0=AOP.add, op1=AOP.max)
                nc.vector.tensor_scalar(out=dc, in0=f, scalar1=1.0,
                                        scalar2=-EPS, op0=AOP.add, op1=AOP.min)
                nc.vector.tensor_scalar(out=msk, in0=f, scalar1=1.0,
                                        scalar2=0.0, op0=AOP.add, op1=AOP.is_ge)
                nc.vector.copy_predicated(dc, msk, t1)
                nc.vector.reciprocal(rc, dc)
                nc.vector.tensor_mul(f, at[:, :, :, kk], rc)
            nc.vector.tensor_add(zt, at[:, :, :, 0], f)
            for tt in range(NTT):
                pz = psB.tile([L, 128], F32, name=f"pz{tt}", tag="pz")
                nc.tensor.transpose(pz, zt[:, tt, :], ident)
                nc.vector.tensor_copy(zT32[:, tt * 128:(tt + 1) * 128], pz)
            z_dram = drp.tile([L, TPC], F32, tag="zdram")
            nc.sync.dma_start(out=z_dram, in_=zT32)

        # ---------------- Phase C: out = x @ U_w.T + z @ V_w.T -------------
        # U as 3 fp8 DoubleRow passes (x8*u8 + x8*ul8 + xu8*u8, all at scale
        # 16); the K=3 V contraction runs as exact-fp32 DVE FMAs against
        # broadcast z rows (V*16 to match the U psum scale), fused with the
        # psum drain.  Three U groups run ahead of the z transposes so the
        # PE never idles on the continued-fraction tail.
        with tc.tile_pool(name="psC", bufs=1, space="PSUM") as psC, \
             tc.tile_pool(name="ob", bufs=2) as obp, \
             tc.tile_pool(name="zb", bufs=1) as zbp:
            zbc = [zbp.tile([128, TPC], F32, name=f"zbc{l}", tag=f"zbc{l}")
                   for l in range(L)]
            def emit_u(dt):
                u8t, ul8t = u_tiles.pop(dt) if dt in u_tiles else load_u(dt)
                po = [psC.tile([128, 512], F32, name=f"po{dt}_{t}",
                               tag=f"po{t}", bufs=4) for t in range(2)]
                for k in range(0, KT, 2):
                    for t in range(2):
                        sl = slice(t * 512, (t + 1) * 512)
                        nc.tensor.matmul(po[t], u8t[:, k:k + 2, :], pair(x8, k, sl),
                                         start=(k == 0), stop=False, perf_mode=DR)
                        nc.tensor.matmul(po[t], ul8t[:, k:k + 2, :], pair(x8, k, sl),
                                         start=False, stop=False, perf_mode=DR)
                        nc.tensor.matmul(po[t], u8t[:, k:k + 2, :], pair(xu8, k, sl),
                                         start=False, stop=(k == KT - 2),
                                         perf_mode=DR)
                return po

            def emit_epi(dt, po, nq=2):
                o32 = obp.tile([128, TPC], F32, name=f"o32_{dt}", tag="o32")
                w = TPC // nq
                for q in range(nq):
                    sl = slice(q * w, (q + 1) * w)
                    psl = slice((q * w) % 512, (q * w) % 512 + w)
                    nc.vector.scalar_tensor_tensor(
                        out=o32[:, sl], in0=zbc[0][:, sl],
                        scalar=vw[:, dt, 0:1], in1=po[q * 2 // nq][:, psl],
                        op0=AOP.mult, op1=AOP.add)
                    for l in range(1, L):
                        nc.vector.scalar_tensor_tensor(
                            out=o32[:, sl], in0=zbc[l][:, sl],
                            scalar=vw[:, dt, l:l + 1], in1=o32[:, sl],
                            op0=AOP.mult, op1=AOP.add)
                    eng = nc.scalar if q % 2 == 0 else nc.sync
                    eng.dma_start(out=d_out[dt * 128:(dt + 1) * 128, sl],
                                  in_=o32[:, sl])
            po_q = [emit_u(0), emit_u(1)]
            for l in range(L):
                eng = nc.scalar if l == 1 else nc.sync
                eng.dma_start(
                    out=zbc[l],
                    in_=z_dram[l:l + 1, :].to_broadcast([128, TPC]))
            po_q += [emit_u(2), emit_u(3)]
            vpre = {}

            def emit_vpre(dt):
                ov = obp.tile([128, TPC], F32, name=f"ov{dt}", tag=f"ov{dt % 2}",
                              bufs=1)
                for t in range(2):
                    sl = slice(t * 512, (t + 1) * 512)
                    nc.vector.tensor_scalar_mul(ov[:, sl], zbc[0][:, sl],
                                                vw[:, dt, 0:1])
                    for l in range(1, L):
                        nc.vector.scalar_tensor_tensor(
                            out=ov[:, sl], in0=zbc[l][:, sl],
                            scalar=vw[:, dt, l:l + 1], in1=ov[:, sl],
                            op0=AOP.mult, op1=AOP.add)
                vpre[dt] = ov
                return ov

            def emit_epi_add(dt, po):
                ov = vpre[dt]
                o32 = obp.tile([128, TPC], F32, name=f"o32_{dt}", tag="o32")
                for t in range(2):
                    sl = slice(t * 512, (t + 1) * 512)
                    nc.vector.tensor_add(o32[:, sl], ov[:, sl], po[t])
                    eng = nc.scalar if t == 0 else nc.sync
                    eng.dma_start(out=d_out[dt * 128:(dt + 1) * 128, sl],
                                  in_=o32[:, sl])

            for dt in range(NDT):
                if dt < NDT - 2:
                    emit_epi(dt, po_q[dt])
                else:
                    emit_epi_add(dt, po_q[dt])
                if dt + 4 < NDT:
                    po_q.append(emit_u(dt + 4))
                if dt == 8:
                    emit_vpre(NDT - 2)
                    emit_vpre(NDT - 1)

    nc.finalize()
    return nc


_NC_CACHE = {}


def _get_program():
    if "nc" not in _NC_CACHE:
        _NC_CACHE["nc"] = _build_program()
    return _NC_CACHE["nc"]


def make_in_maps(x, U_w, gate_w, ladder_w, V_w):
    """Host-side sharding + layout prep. Returns per-core input dicts."""
    x2 = np.ascontiguousarray(np.asarray(x, dtype=np.float32).reshape(TOKENS, D))

    def wtiles(w):
        # w: [out, in] fp32 -> tiles [dt][p][k][o] with
        # tile[dt, p, k, o] = w[dt*128+o, k*128+p]
        a = np.ascontiguousarray(w.T).reshape(KT, 128, NDT, 128)
        return np.ascontiguousarray(a.transpose(2, 1, 0, 3))

    def xtiles(m):
        # m: [D, TPC] -> [128, KT, TPC] with tile[p, k, c] = m[k*128+p, c]
        return np.ascontiguousarray(
            m.reshape(KT, 128, TPC).transpose(1, 0, 2))

    U_w = np.asarray(U_w, np.float32)
    gate_w = np.asarray(gate_w, np.float32)
    ladder_w = np.asarray(ladder_w, np.float32)
    V_w = np.asarray(V_w, np.float32)

    ghi32 = gate_w.astype(np.float16).astype(np.float32)
    ghi_t = wtiles(ghi32).astype(np.float16)
    wl8_t = wtiles((gate_w - ghi32) * S15).astype(E4)

    u16 = U_w * C16
    u8_32 = u16.astype(E4).astype(np.float32)
    u8_t = wtiles(u16).astype(E4)
    ul8_t = wtiles(u16 - u8_32).astype(E4)

    lwT = np.ascontiguousarray(
        ladder_w.transpose(2, 0, 1).reshape(D, LK))     # [d, (l k)]
    lhi32 = lwT.astype(np.float16).astype(np.float32)

    def ltiles(m):
        return np.ascontiguousarray(m.reshape(KT, 128, LK).transpose(1, 0, 2))

    def pad32(a):                                   # [128, KT, 18] -> [128, KT, 32]
        out = np.zeros((128, KT, 32), a.dtype)
        out[:, :, :LK] = a
        return out

    lwhi_t = ltiles(lhi32).astype(np.float16)
    lh8_t = pad32(ltiles(lhi32 * C16).astype(E4))
    ll8_t = pad32(ltiles((lwT - lhi32) * S15).astype(E4))

    vw_t = np.ascontiguousarray(
        (V_w * C16).reshape(NDT, 128, L).transpose(1, 0, 2)).astype(np.float32)

    in_maps = []
    for c in range(NCORES):
        shard = x2[c * TPC:(c + 1) * TPC]              # [TPC, D]
        xT = np.ascontiguousarray(shard.T)             # [D, TPC]
        xhi32 = xT.astype(np.float16).astype(np.float32)
        x8_32 = xT.astype(E4).astype(np.float32)
        in_maps.append({
            "xhi": xtiles(xhi32).astype(np.float16),
            "x8": xtiles(xT).astype(E4),
            "xlo8": xtiles((xT - xhi32) * S11).astype(E4),
            "xu8": xtiles(xT - x8_32).astype(E4),
            "ghi": ghi_t, "wl8": wl8_t,
            "u8": u8_t, "ul8": ul8_t,
            "lwhi": lwhi_t, "lh8": lh8_t, "ll8": ll8_t, "vw": vw_t,
        })
    return in_maps


def assemble_output(results):
    parts = [results[c]["outT"].T for c in range(NCORES)]   # [TPC, D] each
    out = np.concatenate(parts, axis=0)                      # [TOKENS, D]
    return np.ascontiguousarray(
        (out.reshape(4, 2048, D) * (1.0 / C16)).astype(np.float32))


def kernel(x, U_w, gate_w, ladder_w, V_w):
    nc = _get_program()
    in_maps = make_in_maps(x, U_w, gate_w, ladder_w, V_w)
    res = run_bass_kernel_spmd(nc, in_maps, list(range(NCORES)))
    return assemble_output(res.results)


if __name__ == "__main__":
    rng = np.random.default_rng(0)
    x = rng.normal(0, 1, (4, 2048, D)).astype(np.float32)
    s = 1.0 / np.sqrt(D)
    U_w = rng.uniform(-s, s, (D, D)).astype(np.float32)
    gate_w = rng.uniform(-s, s, (D, D)).astype(np.float32)
    ladder_w = rng.uniform(-s, s, (L, DEPTH + 1, D)).astype(np.float32)
    V_w = rng.uniform(-1 / np.sqrt(L), 1 / np.sqrt(L), (D, L)).astype(np.float32)
    out = kernel(x=x, U_w=U_w, gate_w=gate_w, ladder_w=ladder_w, V_w=V_w)
    print("out", out.shape, out.dtype, np.abs(out).max())
